# revision 28
# baseline (speedup 1.0000x reference)
"""Trainium2 Bass kernel for nn_patch_expanding.

Computes, for x [32, 1024, 1024] and w [512, 512]:
    xg = x.reshape(B, 32, 32, 1024); x0, x1 = split(xg, channel halves)
    xi = row-interleave(x0, x1) -> [B, 64, 32, 512]
    y  = xi @ w -> reshape [B, 2048, 512]

Strategy: data-parallel over batch (4 batches/core on 8 cores). Per core the
op is a [4096, 1024] -> [8192, 512] GEMM (contraction over cin=512 per output
row, both channel halves sharing w) plus a row permutation that is folded into
the PE-transpose eviction access pattern. The contraction must sit on SBUF
partitions, so x tiles are transposed on the tensor engine (fp32 transpose
mode), rounded to fp32r during PSUM eviction on DVE, and fed as stationary
operands to full-rate fp32r matmuls with w moving (N=512).

v2 pipeline (vs the original): startup barrier is a gpsimd GO semaphore
instead of 20us of dead-wait NOPs; the transpose identity is shipped as an
extra DRAM input (DMA'd, not built on the slow gpsimd); w/ident load on the
scalar-engine queue so x group loads start immediately; y tiles are stored
STRAIGHT FROM PSUM per tile (no ACT eviction, no SBUF y staging), which
collapses the drain tail; x group loads are quad-buffered to absorb HBM
load/store contention jitter.
"""
import sys
sys.path.insert(0, "/opt/trn_rl_repo")
import numpy as np

B, L, C = 32, 1024, 1024
NCORES = 8
BPC = B // NCORES          # batches per core
ROWS = BPC * L             # 4096 x-rows per core
OROWS = 2 * ROWS           # 8192 y-rows per core
NDB = ROWS // 128          # 32 pipeline tiles per core
G = 4                      # tiles per DMA group (2 MB loads)
NG = NDB // G
LDEPTH = 4                 # resident x load groups (quad buffer)
STAIL = 6                  # trailing store groups issued from the sync queue

_CACHE = {}


def _build(reps: int = 1):
    import concourse.bass as bass
    from concourse import mybir

    f32, f32r = mybir.dt.float32, mybir.dt.float32r
    nc = bass.Bass(trn_type="TRN2", target_bir_lowering=False, debug=False,
                   num_devices=NCORES)

    xd = nc.dram_tensor("x", [ROWS, C], f32, kind="ExternalInput").ap()
    # host-prepped [128, 2176]: per partition p the 4 w rows {128k+p} (2048)
    # then ident row p (128) -- one 8.5KB-descriptor-per-partition DMA
    wdd = nc.dram_tensor("wid", [128, 2176], f32, kind="ExternalInput").ap()
    yd = nc.dram_tensor("y", [OROWS, 512], f32, kind="ExternalOutput").ap()

    # NOTE: completion increments of concurrently in-flight DMAs interleave on
    # a shared sem (16 per-engine +1s each), so a threshold only implies a
    # specific transfer completed if at most one transfer is in flight per
    # sem. Loads cycle over LDEPTH sems (one per buffer slot; the s_tr gate
    # keeps each slot's reload behind its previous consumption), stores over
    # buffer parity.
    s_go = nc.alloc_semaphore("s_go")    # gpsimd finished clearing sems
    s_ld = [nc.alloc_semaphore(f"s_ld{i}") for i in range(LDEPTH)]
    s_f = [nc.alloc_semaphore(f"s_f{i}") for i in range(G)]  # group-0 tile fills
    s_lw = nc.alloc_semaphore("s_lw")    # w+ident load
    s_tr = nc.alloc_semaphore("s_tr")    # PE transposes done, +1 per tile
    s_xt = nc.alloc_semaphore("s_xt")    # DVE xt evictions done, +1 per tile
    s_mm = nc.alloc_semaphore("s_mm")    # PE matmuls done, +1 per tile
    s_ye = nc.alloc_semaphore("s_ye")    # ACT psum evictions done, +1 per tile
    s_yd = nc.alloc_semaphore("s_yd")    # store group flushed (ACT drain), +1
    s_st = [nc.alloc_semaphore("s_st0"), nc.alloc_semaphore("s_st1")]
    s_w = nc.alloc_semaphore("s_w")      # w rounded to fp32r
    all_sems = ([s_go] + s_ld + s_f + s_st +
                [s_lw, s_tr, s_xt, s_mm, s_ye, s_yd, s_w])

    T = NDB * reps

    with (
        nc.sbuf_tensor("xin", [128, LDEPTH, G, 1024], f32) as xin,
        nc.sbuf_tensor("xt", [128, 2, 4, 256], f32) as xt,
        nc.sbuf_tensor("wsb", [128, 2176], f32) as wsb,
        nc.sbuf_tensor("wr", [128, 4, 512], f32) as wr,
        nc.sbuf_tensor("yo", [128, 2, 4, 512], f32) as yo,
        nc.psum_tensor("tp", [128, 4, 512], f32) as tp,
        nc.psum_tensor("mm", [128, 4, 512], f32) as mm,
    ):
        xin_a, xt_a, wsb_a, wr_a = xin.ap(), xt.ap(), wsb.ap(), wr.ap()
        yo_a, tp_a, mm_a = yo.ap(), tp.ap(), mm.ap()
        wv_a = wsb_a[:, :2048].rearrange("p (kk n) -> p kk n", kk=4)
        id_a = wsb_a[:, 2048:]
        # xt viewed with the (d, s, w32) row split used by the evict scatter
        xt_v = xt_a.rearrange("p par kk (d s2 q) -> p par kk d s2 q", d=4, s2=2, q=32)

        # sems are NOT guaranteed zero at kernel entry (device state persists
        # across executions and barriers are unreliable in this runtime).
        # gpsimd clears them all (s_go first) then raises s_go; every other
        # engine sits in a short dead-wait that outlasts the s_go clear, then
        # blocks on s_go >= 1 which it can only see post-clears.
        for s in all_sems:
            nc.gpsimd.sem_clear(s)
        nc.gpsimd.drain().then_inc(s_go)
        for eng in (nc.sync, nc.tensor, nc.vector, nc.scalar):
            eng.nop(cycle_cnt=1500, nofuse=True)
            eng.wait_ge(s_go, 1)

        with nc.Block() as block:

            @block.gpsimd
            def _(g):
                # do not let the program end before the last store lands, and
                # leave the sems clean for the next execution
                g.wait_ge(s_st[0], 16 * (T // 4))
                g.wait_ge(s_st[1], 16 * (T // 4))
                for s in all_sems:
                    g.sem_clear(s)

            @block.sync
            def _(sp):
                # group 0 split per tile so the pipeline fills as each 512KB
                # lands; later groups are single 2MB DMAs, quad-buffered
                for o in range(G):
                    sp.dma_start(
                        xin_a[:, 0, o, :],
                        xd[128 * o:128 * o + 128, :],
                    ).then_inc(s_f[o], 16)
                for gg in range(1, NG * reps):
                    b, ga = gg % LDEPTH, gg % NG
                    if gg >= LDEPTH:
                        sp.wait_ge(s_tr, 4 * (gg - LDEPTH) + 4)  # xin[b] free
                    sp.dma_start(
                        xin_a[:, b, :, :],
                        xd[512 * ga:512 * ga + 512, :].rearrange(
                            "(o p) c -> p o c", p=128),
                    ).then_inc(s_ld[b], 16)
                # tail stores: after loads wind down, drain the last store
                # groups on this (otherwise idle) DMA queue in parallel with
                # the scalar queue
                for sg in range(max(0, T // 2 - STAIL), T // 2):
                    sgpar, sga = sg % 2, sg % (NDB // 2)
                    sp.wait_ge(s_yd, sg + 1)              # yo[sgpar] flushed
                    sp.dma_start(
                        yd[512 * sga:512 * sga + 512, :].rearrange(
                            "(o p) n -> p o n", p=128),
                        yo_a[:, sgpar, :, :],
                    ).then_inc(s_st[sgpar], 16)

            @block.scalar
            def _(ac):
                # w+ident load (one 128-descriptor DMA) on the scalar queue so
                # sync's x loads start immediately
                ac.dma_start(wsb_a[:], wdd).then_inc(s_lw, 16)
                # per-tile PSUM eviction; stores batched per 2 tiles (1 MB) to
                # amortize DMA issue/ring overhead while keeping the tail
                # short; the last STAIL store groups are issued by sync
                for t in range(T):
                    par = t % 2
                    sg, u = t // 2, t % 2      # store group of 2 tiles
                    sgpar = sg % 2
                    ac.wait_ge(s_mm, t + 1)               # mm[par] filled
                    if u == 0 and sg >= 2:
                        ac.wait_ge(s_st[sgpar], 16 * (sg // 2))  # yo[sgpar] free
                    ac.copy(yo_a[:, sgpar, 2 * u:2 * u + 2, :],
                            mm_a[:, 2 * par:2 * par + 2, :]).then_inc(s_ye)
                    if u == 1:
                        sga = sg % (NDB // 2)
                        ac.drain().then_inc(s_yd)
                        if sg < T // 2 - STAIL:
                            ac.dma_start(
                                yd[512 * sga:512 * sga + 512, :].rearrange(
                                    "(o p) n -> p o n", p=128),
                                yo_a[:, sgpar, :, :],
                            ).then_inc(s_st[sgpar], 16)

            @block.tensor
            def _(pe):
                for it in range(T + 1):
                    if it < T:
                        t, par = it, it % 2
                        gg, o = t // G, t % G
                        b = gg % LDEPTH
                        if t == 0:
                            pe.wait_ge(s_lw, 16)          # ident loaded
                        if gg == 0:
                            pe.wait_ge(s_f[o], 16)        # fill tile o loaded
                        elif o == 0:
                            pe.wait_ge(s_ld[b], 16 * (gg // LDEPTH +
                                                      (1 if b else 0)))
                        # tp[par] free: covered by MM(it-2)'s s_xt wait
                        for s in (0, 1):
                            for kk in range(4):
                                inst = pe.matmul(
                                    tp_a[:, 2 * par + s, 128 * kk:128 * kk + 128],
                                    xin_a[:, b, o, 512 * s + 128 * kk:512 * s + 128 * kk + 128],
                                    id_a[:],
                                    is_transpose=True,
                                    start=(kk == 0), stop=(kk == 3),
                                )
                                if (s, kk) == (1, 3):
                                    inst.then_inc(s_tr)
                    if it >= 1:
                        t, par = it - 1, (it - 1) % 2
                        if t == 0:
                            pe.wait_ge(s_w, 1)            # w rounded
                        pe.wait_ge(s_xt, t + 1)           # xt[par] ready
                        if t >= 2:
                            pe.wait_ge(s_ye, t - 1)       # mm[par] free
                        for blk in (0, 1):
                            for kk in range(4):
                                inst = pe.matmul(
                                    mm_a[:, 2 * par + blk, :],
                                    xt_a[:, par, kk, 128 * blk:128 * blk + 128].bitcast(f32r),
                                    wr_a[:, kk, :].bitcast(f32r),
                                    start=(kk == 0), stop=(kk == 3),
                                )
                                if (blk, kk) == (1, 3):
                                    inst.then_inc(s_mm)

            @block.vector
            def _(dv):
                dv.wait_ge(s_lw, 16)
                dv.tensor_copy(wr_a[:].bitcast(f32r), wv_a)
                dv.drain().then_inc(s_w)
                for t in range(T):
                    par = t % 2
                    dv.wait_ge(s_tr, t + 1)               # tp[par] filled
                    if t >= 2:
                        dv.wait_ge(s_mm, t - 1)           # xt[par] free
                    dv.tensor_copy(
                        xt_v[:, par].transpose([0, 3, 1, 2, 4]).bitcast(f32r),
                        tp_a[:, 2 * par:2 * par + 2, :].rearrange(
                            "p s2 (kk d q) -> p s2 kk d q", kk=4, d=4, q=32),
                    )
                    dv.drain().then_inc(s_xt)

    return nc


def _pack_wid(w: np.ndarray) -> np.ndarray:
    """[128, 2176]: partition p holds w rows {128k+p, k=0..3} then ident row p."""
    wk = np.ascontiguousarray(
        w.reshape(4, 128, 512).transpose(1, 0, 2).reshape(128, 2048))
    return np.concatenate([wk, np.eye(128, dtype=np.float32)], axis=1)


def kernel(x: np.ndarray, w: np.ndarray) -> np.ndarray:
    from concourse.bass_utils import run_bass_kernel_spmd

    if "nc" not in _CACHE:
        _CACHE["nc"] = _build()
    nc = _CACHE["nc"]

    x = np.ascontiguousarray(x, dtype=np.float32)
    w = np.ascontiguousarray(w, dtype=np.float32)
    wid = _pack_wid(w)
    xs = x.reshape(NCORES, ROWS, C)
    in_maps = [{"x": xs[i], "wid": wid} for i in range(NCORES)]
    res = run_bass_kernel_spmd(nc, in_maps, list(range(NCORES)))
    y = np.stack([res.results[i]["y"] for i in range(NCORES)], axis=0)
    return y.reshape(B, 2 * L, C // 2)
